# revision 16
# baseline (speedup 1.0000x reference)
"""Trainium2 Bass kernel for the DNNPerf GNN message-passing model (8 cores).

Math restructure (exact up to fp reassociation):
  hpre = h @ W_u ; h' = lrelu(hpre) ; g = lrelu(lrelu(hpre))
  u_n = h'_n . a[:128] ; v_n = h'_n . a[128:]
  t_i = e_i . (W_e @ W_m)                      (per-edge scalar)
  z_i = sigmoid(u[src_i] + v[dst_i]) * t_i
  S = sum_i exp(z_i) ; w_n = sum_{src_i=n} exp(z_i)
  hg = (sum_n w_n * g_n) / S                   (segment_sum + node-sum collapse)
  out = MLP(hg)

Sharding: edges sorted by src, sharded at node-aligned boundaries (edge counts
~equal). Per core the edge stream is [128 rows x C cols]; nodes are dealt to
partitions degree-sorted so every 16-partition group shares one run-length
pattern (lets GPSIMD ap_gather use its group-shared index streams).
"""

import os
import sys
import types

import numpy as np

sys.path.insert(0, "/opt/trn_rl_repo")

import ml_dtypes  # noqa: E402

import concourse.bass as bass  # noqa: E402
import concourse.bacc as bacc  # noqa: E402
import concourse.mybir as mybir  # noqa: E402
from concourse.tile import TileContext  # noqa: E402
from concourse.bass_utils import run_bass_kernel_spmd  # noqa: E402
from concourse.masks import make_identity  # noqa: E402

AF = mybir.ActivationFunctionType
ALU = mybir.AluOpType
F32 = mybir.dt.float32
BF16 = mybir.dt.bfloat16
I16 = mybir.dt.int16

N_NODES = 50000
N_EDGES = 600000
D = 128
DE = 64
NCORES = 8
ALPHA = 0.01

P = 128
NJ = 52
NPN = P * NJ          # 6656 padded nodes per core
NPALL = NCORES * NPN  # 53248
JCAP = 64
C = 704               # stream columns per partition row
CHUNKS = [(0, 512), (512, C - 512)]

_CACHE = {}
LAST_EXEC_NS = [None]


# ====================================================================
# Host-side index prep (pure index logistics; no model math)
# ====================================================================
def _host_prep(src, dst):
    src = np.asarray(src).astype(np.int64)
    dst = np.asarray(dst).astype(np.int64)
    E = src.shape[0]

    order = np.argsort(src, kind="stable")
    src_s = src[order]
    deg_all = np.bincount(src, minlength=N_NODES)
    cum = np.cumsum(deg_all)

    targets = (np.arange(1, NCORES) * E) // NCORES
    nb = [0] + [int(np.searchsorted(cum, t)) + 1 for t in targets] + [N_NODES]
    eb = [0] + [int(cum[b - 1]) for b in nb[1:-1]] + [E]

    per_core = []
    pad_node_of = np.zeros(N_NODES, dtype=np.int64)
    for c in range(NCORES):
        n0, n1 = nb[c], nb[c + 1]
        e0, e1 = eb[c], eb[c + 1]
        n_c = n1 - n0
        assert n_c <= NPN, f"core {c}: {n_c} nodes > {NPN}"
        assert e1 - e0 <= P * C, f"core {c}: {e1 - e0} edges > {P * C}"
        deg = deg_all[n0:n1]

        node_ids = np.full(NPN, -1, dtype=np.int64)
        node_ids[:n_c] = np.arange(n0, n1)
        degp = np.zeros(NPN, dtype=np.int64)
        degp[:n_c] = deg

        rank = np.argsort(-degp, kind="stable")
        assign_p = np.zeros(NPN, dtype=np.int64)
        assign_j = np.zeros(NPN, dtype=np.int64)
        for j in range(NJ):
            blk = rank[j * P:(j + 1) * P]
            assign_p[blk] = np.arange(P)
            assign_j[blk] = j

        deg_pj = np.zeros((P, NJ), dtype=np.int64)
        nid_pj = np.full((P, NJ), -1, dtype=np.int64)
        deg_pj[assign_p, assign_j] = degp
        nid_pj[assign_p, assign_j] = node_ids

        colw = np.zeros((8, NJ), dtype=np.int64)
        for g in range(8):
            colw[g] = deg_pj[16 * g:16 * g + 16, :].max(axis=0)
        startc = np.zeros((8, NJ + 1), dtype=np.int64)
        startc[:, 1:] = np.cumsum(colw, axis=1)
        assert startc[:, -1].max() <= C, (
            f"core {c}: packed width {startc[:, -1].max()} > C={C}")

        valid = nid_pj >= 0
        gid = NPN * c + np.arange(P)[:, None] * NJ + np.arange(NJ)[None, :]
        pad_node_of[nid_pj[valid]] = gid[valid]

        per_core.append(dict(n0=n0, n1=n1, e0=e0, e1=e1,
                             order=order[e0:e1], src_s=src_s[e0:e1],
                             deg_pj=deg_pj, nid_pj=nid_pj,
                             colw=colw, startc=startc))

    def wrap16(streams, ni):
        # streams [8, ni] -> [128, ni//16] per-group 16-partition wrap
        iv = np.zeros((P, ni // 16), np.int16)
        for g in range(8):
            iv[16 * g:16 * g + 16, :] = (
                streams[g].astype(np.int16).reshape(ni // 16, 16).T)
        return iv

    for c in range(NCORES):
        pc = per_core[c]
        deg_pj, nid_pj, startc = pc["deg_pj"], pc["nid_pj"], pc["startc"]
        dst_c = dst[pc["order"]]
        src_c = pc["src_s"]
        first = np.searchsorted(src_c, np.arange(pc["n0"], pc["n1"]))

        slot_edge = np.full((P, C), -1, dtype=np.int64)
        for pp in range(P):
            g = pp // 16
            for j in range(NJ):
                d_ = deg_pj[pp, j]
                if d_ == 0:
                    continue
                f = first[nid_pj[pp, j] - pc["n0"]]
                c0 = startc[g, j]
                slot_edge[pp, c0:c0 + d_] = np.arange(f, f + d_)

        mask = slot_edge >= 0
        se = np.where(mask, slot_edge, 0)
        dstp = np.where(mask, pad_node_of[dst_c[se]], 0)

        idx_u = np.full((8, C), JCAP - 1, dtype=np.int64)
        for g in range(8):
            for j in range(NJ):
                idx_u[g, startc[g, j]:startc[g, j + 1]] = j
        idx_e = np.zeros((8, JCAP), dtype=np.int64)
        for g in range(8):
            ends = np.maximum(startc[g, 1:] - 1, 0)
            idx_e[g, :NJ] = ends
            idx_e[g, NJ:] = ends[-1]

        # v-gather: 4 batched instructions; batch b serves rows 16g+4b..+3
        idx_v = np.zeros((P, C), np.int16)
        BW = 4 * C // 16
        for b in range(4):
            streams = np.zeros((8, 4 * C), np.int64)
            for kk in range(4):
                k = 4 * b + kk
                streams[:, kk * C:(kk + 1) * C] = dstp[k::16, :] >> 1
            idx_v[:, b * BW:(b + 1) * BW] = wrap16(streams, 4 * C)

        pc["idx_v"] = idx_v
        pc["oddmask"] = (dstp & 1).astype(np.float32)
        pc["idx_u16"] = wrap16(idx_u, C)
        pc["idx_e16"] = wrap16(idx_e, JCAP)
        pc["mask_f"] = mask.astype(np.float32)
        pc["dst_c"] = dst_c
        pc["slot_edge"] = slot_edge
    rowsel = np.zeros((P, 16), np.float32)
    rowsel[np.arange(P), np.arange(P) % 16] = 1.0
    return per_core, rowsel


def _host_arrays(per_core, h, e):
    h = np.asarray(h, dtype=np.float32)
    e = np.asarray(e, dtype=np.float32)
    outs = []
    for pc in per_core:
        hsh = np.zeros((NPN, D), np.float32)
        nid = pc["nid_pj"]
        valid = nid >= 0
        rows = (np.arange(P)[:, None] * NJ + np.arange(NJ)[None, :])[valid]
        hsh[rows] = h[nid[valid]]

        esl = np.zeros((P, C, DE), np.float32)
        m = pc["mask_f"].astype(bool)
        esl[m] = e[pc["order"][pc["slot_edge"][m]]]

        blocks = []
        for (c0, n) in CHUNKS:
            for j in range(64):
                top = esl[2 * j, c0:c0 + n, :].T
                bot = esl[2 * j + 1, c0:c0 + n, :].T
                blocks.append(np.concatenate([top, bot], axis=0))
        eT = np.ascontiguousarray(np.concatenate(blocks, axis=1))
        outs.append(dict(h_shard=hsh, eT=eT))
    return outs


# ====================================================================
# Device program
# ====================================================================
def _build_program(stage=99):
    nc = bacc.Bacc(trn_type="TRN2", num_devices=NCORES)

    h_shard = nc.dram_tensor("h_shard", [NPN, D], F32, kind="ExternalInput")
    eT = nc.dram_tensor("eT", [P, 64 * C], F32, kind="ExternalInput")
    mask_d = nc.dram_tensor("mask", [P, C], F32, kind="ExternalInput")
    oddm_d = nc.dram_tensor("oddmask", [P, C], F32, kind="ExternalInput")
    rowsel_d = nc.dram_tensor("rowsel", [P, 16], F32, kind="ExternalInput")
    idxv_d = nc.dram_tensor("idx_v", [P, C], I16, kind="ExternalInput")
    idxu_d = nc.dram_tensor("idx_u", [P, C // 16], I16, kind="ExternalInput")
    idxe_d = nc.dram_tensor("idx_e", [P, JCAP // 16], I16, kind="ExternalInput")
    W_u = nc.dram_tensor("W_u", [D, D], F32, kind="ExternalInput")
    a_d = nc.dram_tensor("a", [2 * D, 1], F32, kind="ExternalInput")
    W_e = nc.dram_tensor("W_e", [DE, D], F32, kind="ExternalInput")
    W_m = nc.dram_tensor("W_m", [D, 1], F32, kind="ExternalInput")
    W1 = nc.dram_tensor("W1", [D, 512], F32, kind="ExternalInput")
    b1 = nc.dram_tensor("b1", [1, 512], F32, kind="ExternalInput")
    W2 = nc.dram_tensor("W2", [512, D], F32, kind="ExternalInput")
    b2 = nc.dram_tensor("b2", [1, D], F32, kind="ExternalInput")
    W3 = nc.dram_tensor("W3", [D, 16], F32, kind="ExternalInput")
    b3 = nc.dram_tensor("b3", [1, 16], F32, kind="ExternalInput")
    W4 = nc.dram_tensor("W4", [16, 1], F32, kind="ExternalInput")
    b4 = nc.dram_tensor("b4", [1, 1], F32, kind="ExternalInput")
    y = nc.dram_tensor("y", [1, 1], F32, kind="ExternalOutput")

    v_cin = nc.dram_tensor("v_cin", [NPN], BF16, kind="Internal")
    v_all = nc.dram_tensor("v_all", [NPALL], BF16, kind="Internal",
                           addr_space="Shared")
    hgs_in = nc.dram_tensor("hgs_in", [132], F32, kind="Internal")
    hgs_out = nc.dram_tensor("hgs_out", [132], F32, kind="Internal",
                             addr_space="Shared")
    rg = [list(range(NCORES))]

    with TileContext(nc) as tc:
        with (
            tc.tile_pool(name="const", bufs=1) as cp,
            tc.tile_pool(name="node", bufs=2) as npl,
            tc.tile_pool(name="est", bufs=2) as ep,
            tc.tile_pool(name="stream", bufs=1) as sp,
            tc.tile_pool(name="vgp", bufs=2) as vgp,
            tc.tile_pool(name="psA", bufs=2, space="PSUM") as psA,
            tc.tile_pool(name="psB", bufs=2, space="PSUM") as psB,
            tc.tile_pool(name="psT", bufs=2, space="PSUM") as psT,
        ):
            ident = cp.tile([P, P], F32)
            make_identity(nc, ident[:])
            ones_row = cp.tile([1, P], F32)
            nc.vector.memset(ones_row[:], 1.0)
            ones_col = cp.tile([P, 1], F32)
            nc.vector.memset(ones_col[:], 1.0)

            # ---------- small weights ----------
            wu_t = cp.tile([D, D], F32)
            nc.sync.dma_start(out=wu_t[:], in_=W_u[:])
            a1_t = cp.tile([D, 1], F32)
            nc.sync.dma_start(out=a1_t[:], in_=a_d[0:D, :])
            a2_t = cp.tile([D, 1], F32)
            nc.sync.dma_start(out=a2_t[:], in_=a_d[D:2 * D, :])
            we_t = cp.tile([DE, D], F32)
            nc.sync.dma_start(out=we_t[:], in_=W_e[:])
            wm_t = cp.tile([D, 1], F32)
            nc.sync.dma_start(out=wm_t[:], in_=W_m[:])

            weT_p = psA.tile([D, DE], F32, tag="pss")
            nc.tensor.transpose(out=weT_p[:], in_=we_t[:], identity=ident[0:DE, 0:DE])
            weT_s = cp.tile([D, DE], F32)
            nc.scalar.copy(out=weT_s[:], in_=weT_p[:])
            wem_p = psA.tile([DE, 1], F32, tag="pss")
            nc.tensor.matmul(out=wem_p[:], lhsT=weT_s[:], rhs=wm_t[:],
                             start=True, stop=True)
            lw = cp.tile([P, 320], F32)
            nc.vector.memset(lw[:], 0.0)
            nc.scalar.copy(out=lw[0:DE, 190:191], in_=wem_p[:])
            nc.scalar.copy(out=lw[DE:2 * DE, 191:192], in_=wem_p[:])

            # A1R/A2R: a-vector replicated on every partition row
            a1r_p = psA.tile([1, D], F32, tag="pss")
            nc.tensor.transpose(out=a1r_p[:], in_=a1_t[:], identity=ident[:])
            a1_row = cp.tile([1, D], F32)
            nc.scalar.copy(out=a1_row[:], in_=a1r_p[:])
            a2r_p = psA.tile([1, D], F32, tag="pss")
            nc.tensor.transpose(out=a2r_p[:], in_=a2_t[:], identity=ident[:])
            a2_row = cp.tile([1, D], F32)
            nc.scalar.copy(out=a2_row[:], in_=a2r_p[:])
            A1R_p = psA.tile([P, D], F32, tag="pss")
            nc.tensor.matmul(out=A1R_p[:], lhsT=ones_row[:], rhs=a1_row[:],
                             start=True, stop=True)
            A1R = cp.tile([P, D], F32)
            nc.scalar.copy(out=A1R[:], in_=A1R_p[:])
            A2R_p = psA.tile([P, D], F32, tag="pss")
            nc.tensor.matmul(out=A2R_p[:], lhsT=ones_row[:], rhs=a2_row[:],
                             start=True, stop=True)
            A2R = cp.tile([P, D], F32)
            nc.scalar.copy(out=A2R[:], in_=A2R_p[:])

            # ---------- node stage ----------
            g_store = sp.tile([P, NJ * D], BF16)
            u_own = sp.tile([P, JCAP], F32)
            nc.vector.memset(u_own[:], 0.0)
            v_own = sp.tile([P, NJ], F32)
            h3 = h_shard.ap().rearrange("(p j) d -> p j d", j=NJ)
            if stage < 1:
                nc.vector.memset(v_own[:], 0.001)
                nc.vector.memset(g_store[:], 0.001)
            for j in (range(NJ) if stage >= 1 else []):
                ht = npl.tile([P, D], F32, tag="ht")
                nc.sync.dma_start(out=ht[:], in_=h3[:, j, :])
                hT_p = psB.tile([P, D], F32, tag="hT")
                nc.tensor.transpose(out=hT_p[:], in_=ht[:], identity=ident[:])
                hT_s = npl.tile([P, D], F32, tag="hTs")
                nc.scalar.copy(out=hT_s[:], in_=hT_p[:])
                hpre_p = psB.tile([P, D], F32, tag="hpre")
                nc.tensor.matmul(out=hpre_p[:], lhsT=hT_s[:], rhs=wu_t[:],
                                 start=True, stop=True)
                hpre_s = npl.tile([P, D], F32, tag="hpre_s")
                nc.scalar.copy(out=hpre_s[:], in_=hpre_p[:])
                hp_t = npl.tile([P, D], F32, tag="hp")
                nc.vector.tensor_scalar_mul(hp_t[:], hpre_s[:], ALPHA)
                nc.vector.tensor_max(hp_t[:], hp_t[:], hpre_s[:])
                g_blk = npl.tile([P, D], F32, tag="g_blk")
                nc.vector.tensor_scalar_mul(g_blk[:], hpre_s[:], ALPHA * ALPHA)
                nc.vector.tensor_max(g_blk[:], g_blk[:], hpre_s[:])
                nc.vector.tensor_copy(out=g_store[:, j * D:(j + 1) * D],
                                      in_=g_blk[:])
                scr_u = npl.tile([P, D], F32, tag="scru")
                nc.vector.tensor_mul(scr_u[:], hp_t[:], A1R[:])
                nc.vector.tensor_reduce(out=u_own[:, j:j + 1], in_=scr_u[:],
                                        axis=mybir.AxisListType.X, op=ALU.add)
                scr_v = npl.tile([P, D], F32, tag="scrv")
                nc.vector.tensor_mul(scr_v[:], hp_t[:], A2R[:])
                nc.vector.tensor_reduce(out=v_own[:, j:j + 1], in_=scr_v[:],
                                        axis=mybir.AxisListType.X, op=ALU.add)

            v_bf = sp.tile([P, NJ], BF16)
            nc.vector.tensor_copy(out=v_bf[:], in_=v_own[:])
            nc.sync.dma_start(
                out=v_cin.ap().rearrange("(p j) -> p j", p=P), in_=v_bf[:])
            nc.gpsimd.collective_compute(
                "AllGather", ALU.bypass, ins=[v_cin[:]], outs=[v_all[:]],
                replica_groups=rg)
            vtab = sp.tile([P, NPALL], BF16)
            nc.sync.dma_start(out=vtab[:],
                              in_=bass.AP(v_all, 0, [[0, P], [1, NPALL]]))

            # ---------- t stream via PE ----------
            t_str = sp.tile([P, C], F32)
            if stage < 2:
                nc.vector.memset(t_str[:], 0.001)
            eoff = 0
            for (c0, n) in (CHUNKS if stage >= 2 else []):
                tp = psT.tile([P, 512], F32, tag="tp")
                for j in range(64):
                    rhs = ep.tile([P, 512], F32, tag="rhs")
                    nc.sync.dma_start(out=rhs[:, 0:n], in_=eT[:, eoff:eoff + n])
                    eoff += n
                    nc.tensor.matmul(
                        out=tp[:, 0:n], lhsT=lw[:, 190 - 2 * j:318 - 2 * j],
                        rhs=rhs[:, 0:n], start=(j == 0), stop=(j == 63))
                nc.vector.tensor_copy(out=t_str[:, c0:c0 + n], in_=tp[:, 0:n])

            # ---------- u expansion ----------
            idxu_t = sp.tile([P, C // 16], I16)
            nc.sync.dma_start(out=idxu_t[:], in_=idxu_d[:])
            u_e = sp.tile([P, C], F32)
            if stage >= 3:
                nc.gpsimd.ap_gather(
                    out_ap=u_e[:].rearrange("p (i d) -> p i d", d=1),
                    in_ap=u_own[:].rearrange("p (e d) -> p e d", d=1),
                    idxs_ap=idxu_t[:], channels=P, num_elems=JCAP, d=1,
                    num_idxs=C)
            else:
                nc.vector.memset(u_e[:], 0.001)

            # ---------- v gather (16 sub-gathers + masked accumulate) ----------
            rowsel = cp.tile([P, 16], F32)
            nc.sync.dma_start(out=rowsel[:], in_=rowsel_d[:])
            idxv_t = sp.tile([P, C], I16)
            nc.sync.dma_start(out=idxv_t[:], in_=idxv_d[:])
            acc_ev = sp.tile([P, C], F32)
            nc.vector.memset(acc_ev[:], 0.0)
            acc_od = sp.tile([P, C], F32)
            nc.vector.memset(acc_od[:], 0.0)
            BW = 4 * C // 16
            for b in (range(4) if stage >= 4 else []):
                vg = vgp.tile([P, 8 * C], BF16, tag="vg")
                nc.gpsimd.ap_gather(
                    out_ap=vg[:].rearrange("p (i d) -> p i d", d=2),
                    in_ap=vtab[:].rearrange("p (e d) -> p e d", d=2),
                    idxs_ap=idxv_t[:, b * BW:(b + 1) * BW],
                    channels=P, num_elems=NPALL // 2, d=2, num_idxs=4 * C)
                for kk in range(4):
                    k = 4 * b + kk
                    off = 2 * kk * C
                    vg_ev = vgp.tile([P, C], F32, tag="vg_ev")
                    nc.vector.tensor_copy(out=vg_ev[:],
                                          in_=vg[:, off:off + 2 * C:2])
                    vg_od = vgp.tile([P, C], F32, tag="vg_od")
                    nc.vector.tensor_copy(out=vg_od[:],
                                          in_=vg[:, off + 1:off + 2 * C:2])
                    nc.vector.tensor_scalar(
                        out=vg_ev[:], in0=vg_ev[:], scalar1=rowsel[:, k:k + 1],
                        scalar2=None, op0=ALU.mult)
                    nc.vector.tensor_add(acc_ev[:], acc_ev[:], vg_ev[:])
                    nc.vector.tensor_scalar(
                        out=vg_od[:], in0=vg_od[:], scalar1=rowsel[:, k:k + 1],
                        scalar2=None, op0=ALU.mult)
                    nc.vector.tensor_add(acc_od[:], acc_od[:], vg_od[:])

            oddm = sp.tile([P, C], F32, tag="tmp_c")
            nc.sync.dma_start(out=oddm[:], in_=oddm_d[:])
            # v_e assembled in acc_od; score in u_e; sig in acc_ev; z in
            # acc_od; ez in acc_ev; ezm in t_str (buffer reuse for SBUF).
            nc.vector.tensor_sub(acc_od[:], acc_od[:], acc_ev[:])
            nc.vector.tensor_mul(acc_od[:], acc_od[:], oddm[:])
            nc.vector.tensor_add(acc_od[:], acc_od[:], acc_ev[:])

            # ---------- score / softmax-weight pipeline ----------
            mask_t = sp.tile([P, C], F32)
            nc.sync.dma_start(out=mask_t[:], in_=mask_d[:])
            nc.vector.tensor_add(u_e[:], u_e[:], acc_od[:])
            nc.scalar.activation(acc_ev[:], u_e[:], AF.Sigmoid)
            nc.vector.tensor_mul(acc_od[:], acc_ev[:], t_str[:])
            nc.scalar.activation(acc_ev[:], acc_od[:], AF.Exp)
            nc.vector.tensor_mul(t_str[:], acc_ev[:], mask_t[:])

            ones_c = sp.tile([P, C], F32, tag="tmp_c")
            nc.vector.memset(ones_c[:], 1.0)
            A_t = sp.tile([P, C], F32)
            nc.vector.tensor_tensor_scan(
                out=A_t[:], data0=ones_c[:], data1=t_str[:],
                initial=0.0, op0=ALU.mult, op1=ALU.add)

            idxe_t = sp.tile([P, JCAP // 16], I16)
            nc.sync.dma_start(out=idxe_t[:], in_=idxe_d[:])
            wends = sp.tile([P, JCAP], F32)
            if stage >= 5:
                nc.gpsimd.ap_gather(
                    out_ap=wends[:].rearrange("p (i d) -> p i d", d=1),
                    in_ap=A_t[:].rearrange("p (e d) -> p e d", d=1),
                    idxs_ap=idxe_t[:], channels=P, num_elems=C, d=1,
                    num_idxs=JCAP)
            else:
                nc.vector.memset(wends[:], 0.001)
            w_own = sp.tile([P, NJ], F32)
            nc.vector.tensor_copy(out=w_own[:, 0:1], in_=wends[:, 0:1])
            nc.vector.tensor_sub(w_own[:, 1:NJ], wends[:, 1:NJ],
                                 wends[:, 0:NJ - 1])

            # ---------- readout ----------
            hg_part = sp.tile([P, D], F32)
            nc.vector.memset(hg_part[:], 0.0)
            for j in range(NJ):
                g_f = npl.tile([P, D], F32, tag="g_f")
                nc.vector.tensor_copy(out=g_f[:], in_=g_store[:, j * D:(j + 1) * D])
                nc.vector.tensor_scalar(
                    out=g_f[:], in0=g_f[:], scalar1=w_own[:, j:j + 1],
                    scalar2=None, op0=ALU.mult)
                nc.vector.tensor_add(hg_part[:], hg_part[:], g_f[:])
            hg_p = psA.tile([D, 1], F32, tag="pss")
            nc.tensor.matmul(out=hg_p[:], lhsT=hg_part[:], rhs=ones_col[:],
                             start=True, stop=True)
            hg_col = sp.tile([D, 1], F32)
            nc.vector.tensor_copy(out=hg_col[:], in_=hg_p[:])
            s_p = psA.tile([1, 1], F32, tag="pss")
            nc.tensor.matmul(out=s_p[:], lhsT=wends[:, NJ - 1:NJ],
                             rhs=ones_col[:], start=True, stop=True)
            s_s = sp.tile([1, 1], F32)
            nc.vector.tensor_copy(out=s_s[:], in_=s_p[:])

            nc.sync.dma_start(
                out=hgs_in.ap()[0:D].rearrange("(p o) -> p o", o=1),
                in_=hg_col[:])
            nc.sync.dma_start(
                out=hgs_in.ap()[D:D + 1].rearrange("(p o) -> p o", o=1),
                in_=s_s[:])
            zpad = sp.tile([1, 3], F32)
            nc.vector.memset(zpad[:], 0.0)
            nc.sync.dma_start(
                out=hgs_in.ap()[D + 1:132].rearrange("(o f) -> o f", o=1),
                in_=zpad[:])
            nc.gpsimd.collective_compute(
                "AllReduce", ALU.add, ins=[hgs_in[:]], outs=[hgs_out[:]],
                replica_groups=rg)

            hg2 = sp.tile([D, 1], F32)
            nc.sync.dma_start(
                out=hg2[:], in_=hgs_out.ap()[0:D].rearrange("(p o) -> p o", o=1))
            s2 = sp.tile([1, 1], F32)
            nc.sync.dma_start(
                out=s2[:], in_=hgs_out.ap()[D:D + 1].rearrange("(p o) -> p o", o=1))

            sinv = sp.tile([1, 1], F32)
            nc.vector.reciprocal(out=sinv[:], in_=s2[:])
            sinv_p = psA.tile([P, 1], F32, tag="pss")
            nc.tensor.matmul(out=sinv_p[:], lhsT=ones_row[:], rhs=sinv[:],
                             start=True, stop=True)
            sinv_col = sp.tile([P, 1], F32)
            nc.vector.tensor_copy(out=sinv_col[:], in_=sinv_p[:])
            hgn = sp.tile([D, 1], F32)
            nc.vector.tensor_mul(hgn[:], hg2[:], sinv_col[:])

            # ---------- MLP ----------
            w1_t = cp.tile([D, 512], F32)
            nc.sync.dma_start(out=w1_t[:], in_=W1[:])
            b1_t = cp.tile([1, 512], F32)
            nc.sync.dma_start(out=b1_t[:], in_=b1[:])
            w2_t = cp.tile([P, 4 * D], F32)
            for k in range(4):
                nc.sync.dma_start(out=w2_t[:, k * D:(k + 1) * D],
                                  in_=W2[k * P:(k + 1) * P, :])
            b2_t = cp.tile([1, D], F32)
            nc.sync.dma_start(out=b2_t[:], in_=b2[:])
            w3_t = cp.tile([D, 16], F32)
            nc.sync.dma_start(out=w3_t[:], in_=W3[:])
            b3_t = cp.tile([1, 16], F32)
            nc.sync.dma_start(out=b3_t[:], in_=b3[:])
            w4_t = cp.tile([16, 1], F32)
            nc.sync.dma_start(out=w4_t[:], in_=W4[:])
            b4_t = cp.tile([1, 1], F32)
            nc.sync.dma_start(out=b4_t[:], in_=b4[:])

            x1_p = psA.tile([1, 512], F32, tag="pss")
            nc.tensor.matmul(out=x1_p[:], lhsT=hgn[:], rhs=w1_t[:],
                             start=True, stop=True)
            x1 = sp.tile([1, 512], F32)
            nc.vector.tensor_add(x1[:], x1_p[:], b1_t[:])
            nc.vector.tensor_scalar_max(x1[:], x1[:], 0.0)
            x1T = sp.tile([P, 4], F32)
            for k in range(4):
                xt_p = psA.tile([P, 1], F32, tag="pss")
                nc.tensor.transpose(out=xt_p[:], in_=x1[:, k * P:(k + 1) * P],
                                    identity=ident[0:1, 0:1])
                nc.vector.tensor_copy(out=x1T[:, k:k + 1], in_=xt_p[:])
            x2_p = psA.tile([1, D], F32, tag="pss")
            for k in range(4):
                nc.tensor.matmul(out=x2_p[:], lhsT=x1T[:, k:k + 1],
                                 rhs=w2_t[:, k * D:(k + 1) * D],
                                 start=(k == 0), stop=(k == 3))
            x2 = sp.tile([1, D], F32)
            nc.vector.tensor_add(x2[:], x2_p[:], b2_t[:])
            nc.vector.tensor_scalar_max(x2[:], x2[:], 0.0)
            x2T_p = psA.tile([P, 1], F32, tag="pss")
            nc.tensor.transpose(out=x2T_p[:], in_=x2[:], identity=ident[0:1, 0:1])
            x2T = sp.tile([P, 1], F32)
            nc.vector.tensor_copy(out=x2T[:], in_=x2T_p[:])
            x3_p = psA.tile([1, 16], F32, tag="pss")
            nc.tensor.matmul(out=x3_p[:], lhsT=x2T[:], rhs=w3_t[:],
                             start=True, stop=True)
            x3 = sp.tile([1, 16], F32)
            nc.vector.tensor_add(x3[:], x3_p[:], b3_t[:])
            nc.vector.tensor_scalar_max(x3[:], x3[:], 0.0)
            x3T_p = psA.tile([16, 1], F32, tag="pss")
            nc.tensor.transpose(out=x3T_p[:], in_=x3[:], identity=ident[0:1, 0:1])
            x3T = sp.tile([16, 1], F32)
            nc.vector.tensor_copy(out=x3T[:], in_=x3T_p[:])
            out_p = psA.tile([1, 1], F32, tag="pss")
            nc.tensor.matmul(out=out_p[:], lhsT=x3T[:], rhs=w4_t[:],
                             start=True, stop=True)
            out_s = sp.tile([1, 1], F32)
            nc.vector.tensor_add(out_s[:], out_p[:], b4_t[:])
            nc.sync.dma_start(out=y[:], in_=out_s[:])

    nc.finalize()
    return nc


def _get_program():
    stage = int(os.environ.get("KERNEL_STAGE", "99"))
    if "nc" not in _CACHE:
        _CACHE["nc"] = _build_program(stage)
    return _CACHE["nc"]


# ====================================================================
# Entry point
# ====================================================================
def kernel(h, e, src, dst, W_u, W_e, a, W_m, W1, b1, W2, b2, W3, b3, W4, b4):
    per_core, rowsel = _host_prep(src, dst)
    arrays = _host_arrays(per_core, h, e)

    common = dict(
        rowsel=rowsel,
        W_u=np.asarray(W_u, np.float32),
        a=np.asarray(a, np.float32).reshape(2 * D, 1),
        W_e=np.asarray(W_e, np.float32),
        W_m=np.asarray(W_m, np.float32).reshape(D, 1),
        W1=np.asarray(W1, np.float32),
        b1=np.asarray(b1, np.float32).reshape(1, 512),
        W2=np.asarray(W2, np.float32),
        b2=np.asarray(b2, np.float32).reshape(1, D),
        W3=np.asarray(W3, np.float32),
        b3=np.asarray(b3, np.float32).reshape(1, 16),
        W4=np.asarray(W4, np.float32),
        b4=np.asarray(b4, np.float32).reshape(1, 1),
    )
    in_maps = []
    for c in range(NCORES):
        pc, ar = per_core[c], arrays[c]
        in_maps.append(dict(
            h_shard=ar["h_shard"], eT=ar["eT"],
            mask=pc["mask_f"], oddmask=pc["oddmask"],
            idx_v=pc["idx_v"], idx_u=pc["idx_u16"], idx_e=pc["idx_e16"],
            **common,
        ))

    nc = _get_program()
    trace = bool(os.environ.get("KERNEL_TRACE"))
    if trace:
        try:
            from trn_agent_boot.trn_boot import _ntff_profile_via_ctypes
            hook = _ntff_profile_via_ctypes("/opt/axon/libaxon_pjrt.so")
            mod = types.ModuleType("antenv.axon_hooks")
            mod.get_axon_ntff_profile_hook = lambda: hook
            sys.modules["antenv.axon_hooks"] = mod
        except Exception:
            trace = False
    res = run_bass_kernel_spmd(nc, in_maps, core_ids=list(range(NCORES)),
                               trace=trace)
    LAST_EXEC_NS[0] = res.exec_time_ns
    return np.asarray(res.results[0]["y"], dtype=np.float32)
